# revision 17
# baseline (speedup 1.0000x reference)
"""EpisodicMemory kernel for Trainium2, data-parallel over batch on 8 NeuronCores.

Per-core computation (one batch element b, S=4096, D=1024, M=64, H=4, DH=256):

Host-side algebraic fusion (exact linear algebra in fp64):
  k        = mk @ wk.T + bk                              (M, D)
  FUSED_K  = stack_h[(k_h @ wq_h) / sqrt(DH)]            (H*M, D)
  scores   = x @ FUSED_K.T + sbias        (replaces q-proj + qk matmul)
  BIG_W    = [mk | wg | FUSED_K]                         (M+1+H*M, D)
  fused2   = comb_w[:, D:] @ out_w                       (D, D);  G = fused2.T
  W1T      = comb_w[:, :D].T                             (D, D)
  combb    = comb_b + comb_w[:, D:] @ out_b + bv @ G     (D,)
  final    = x @ W1 + sum_h p_h @ VF_h + combb
  where VF_h = v_h @ G_h,  v = mv @ wv.T  (bv folded into combb since
  sum_m p_hm = 1), i.e. the old mem_out/out_w/comb2 GEMM chain collapses
  into one S x 256 x D matmul against a per-(head,slot) value table VF.

Device phases (per core):
  1. per s-chunk (128 rows): pbig = x_chunk @ BIG_W.T -> [sim | gate | scores]
     softmax(sim)*sigmoid(gate) -> gated -> accumulate W = gated.T @ [x | 1]
     softmax(scores) per head -> p -> PE-transpose -> pT_all (resident)
     No max-subtraction anywhere (|sim|,|gate| < ~5, |scores| < ~0.1 for this
     problem's scale-0.02 weights); sigmoid computed via the Exp table
     (sigmoid(g) = 1/(1+exp(-g))) so the ACT table is loaded exactly once.
  2. slot_gate = min(colsum, 1), mv = slot_gate * W, v = mv @ wv.T,
     VF = blockdiag(v_h) @ G  (256, D)
  3. per s-chunk: out = x_chunk @ W1 + P_chunk @ VF (+ combb) -> DMA out
All matmuls run in bf16 inputs with fp32 PSUM accumulation.
"""

import numpy as np
import ml_dtypes

import concourse.bass as bass
import concourse.mybir as mybir
import concourse.tile as tile
from concourse import bacc
from concourse.bass_utils import run_bass_kernel_spmd
from concourse.masks import make_identity

F32 = mybir.dt.float32
BF16 = mybir.dt.bfloat16
AX = mybir.AxisListType.X
AF = mybir.ActivationFunctionType
ALU = mybir.AluOpType

B, D, M, H = 8, 1024, 64, 4
DH = D // H
GW = M + 1 + H * M  # 321 columns of BIG_W output: [sim 0:64 | gate 64 | scores 65:321]
N_CORES = 8


def build_program(S=4096, add_sbias=False, add_combb=False):
    NCH = S // 128   # s-chunks
    NT = S // 512    # x row-tile groups (4 chunks each)
    DC = D // 128    # d-chunks

    nc = bacc.Bacc(None, target_bir_lowering=False, debug=False)

    # all big inputs are pre-swizzled on the host so every DMA line is
    # contiguous per partition (128 descriptors/transfer, not 1024)
    x_d = nc.dram_tensor("x", [128, NT, 4, D], BF16, kind="ExternalInput")
    xT_d = nc.dram_tensor("xT", [128, NT, DC, 512], BF16, kind="ExternalInput")
    bigwT_d = nc.dram_tensor("bigwT", [128, DC, GW], BF16, kind="ExternalInput")
    w1T_d = nc.dram_tensor("w1T", [128, DC, D], BF16, kind="ExternalInput")
    wvT_d = nc.dram_tensor("wvT", [128, DC, D], BF16, kind="ExternalInput")
    gT_d = nc.dram_tensor("gmat", [128, DC, D], BF16, kind="ExternalInput")
    combb_d = nc.dram_tensor("combb", [D], F32, kind="ExternalInput")
    wgbn_d = nc.dram_tensor("wgbn", [1], F32, kind="ExternalInput")
    sbias_d = nc.dram_tensor("sbias", [H * M], F32, kind="ExternalInput")
    y_d = nc.dram_tensor("y", [S, D], F32, kind="ExternalOutput")

    x_ap = x_d.ap()
    y_ap = y_d.ap()
    xT_r = xT_d.ap()
    bigwT_r = bigwT_d.ap()
    w1T_r = w1T_d.ap()
    wvT_r = wvT_d.ap()
    gT_r = gT_d.ap()

    def bcast(ap, n):
        return bass.AP(tensor=ap.tensor, offset=ap.offset, ap=[[0, n]] + list(ap.ap))

    with tile.TileContext(nc) as tc:
        with tc.tile_pool(name="singles", bufs=1) as singles:
            # resident inputs. bigwT first on the sync queue (first matmul needs
            # it), then xT in 512-column slices as separate tiles so early
            # chunks don't wait on the whole 8.4 MB transfer. wvT/gT/w1T are
            # deferred to just before their consumers (below) so they don't
            # delay the x stream on the sync queue.
            bigwT_sb = singles.tile([128, DC, GW], BF16)
            nc.sync.dma_start(bigwT_sb, bigwT_r)
            xT_t = []
            for t in range(NT):
                xts = singles.tile([128, DC, 512], BF16, tag=f"xts{t}")
                nc.sync.dma_start(xts, xT_r[:, t, :, :])
                xT_t.append(xts)

            def xT_chunk(dc, c):
                return xT_t[c // 4][:, dc, (c % 4) * 128:(c % 4 + 1) * 128]

            wgbn_sb = singles.tile([128, 1], F32)
            nc.gpsimd.dma_start(wgbn_sb, bcast(wgbn_d.ap(), 128))
            if add_sbias:
                sbias_sb = singles.tile([128, H * M], F32)
                nc.gpsimd.dma_start(sbias_sb, bcast(sbias_d.ap(), 128))
            if add_combb:
                combb_sb = singles.tile([128, D], F32)
                nc.gpsimd.dma_start(combb_sb, bcast(combb_d.ap(), 128))
            ident = singles.tile([128, 128], BF16)
            make_identity(nc, ident)
            ones_sb = singles.tile([128, 1], BF16)
            nc.vector.memset(ones_sb, 1.0)
            # residents produced by phase 1 / boundary
            pT_all = singles.tile([128, 2, S], BF16)
            vf_sb = singles.tile([128, 2, D], BF16)

            # ---------------- phase 1: write-attention ----------------
            with (
                tc.tile_pool(name="ps1", bufs=1, space="PSUM") as ps1,
                tc.tile_pool(name="xin", bufs=3) as xin,
                tc.tile_pool(name="wk1", bufs=3) as wk1,
            ):
                ps_w = ps1.tile([64, 1536], F32, tag="w")
                for t in range(NT):
                    xrow = xin.tile([128, 4, D], BF16, tag="xrow")
                    nc.gpsimd.dma_start(xrow, x_ap[:, t, :, :])
                    for c4 in range(4):
                        c = t * 4 + c4
                        pbig = ps1.tile([128, GW], F32, tag="big", bufs=3)
                        for dc in range(DC):
                            nc.tensor.matmul(
                                pbig,
                                lhsT=xT_chunk(dc, c),
                                rhs=bigwT_sb[:, dc, :],
                                start=(dc == 0), stop=(dc == DC - 1),
                            )
                        if add_sbias:
                            nc.vector.tensor_add(
                                pbig[:, M + 1:GW], pbig[:, M + 1:GW], sbias_sb
                            )

                        # --- write gate: softmax(sim) * sigmoid(gate) ---
                        # (no max subtraction; sigmoid via Exp table)
                        esum = wk1.tile([128, 1], F32, tag="esum")
                        e_sb = wk1.tile([128, M], F32, tag="esb")
                        nc.scalar.activation(e_sb, pbig[:, 0:M], AF.Exp,
                                             accum_out=esum)
                        tg = wk1.tile([128, 1], F32, tag="tg")
                        nc.scalar.activation(tg, pbig[:, M:M + 1], AF.Exp,
                                             scale=-1.0, bias=wgbn_sb)
                        u = wk1.tile([128, 1], F32, tag="u")
                        nc.vector.scalar_tensor_tensor(
                            u, tg, 1.0, esum, op0=ALU.add, op1=ALU.mult)
                        scale = wk1.tile([128, 1], F32, tag="scale")
                        nc.vector.reciprocal(scale, u)
                        gc = wk1.tile([128, M], BF16, tag="gc")
                        nc.gpsimd.tensor_scalar_mul(gc, e_sb, scale)

                        # --- read attention probs, per head ---
                        es = wk1.tile([128, H, M], F32, tag="es")
                        nc.scalar.activation(es, pbig[:, M + 1:GW], AF.Exp)
                        dsum = wk1.tile([128, H], F32, tag="dsum")
                        nc.vector.reduce_sum(dsum, es, axis=AX)
                        drec = wk1.tile([128, H], F32, tag="drec")
                        nc.vector.reciprocal(drec, dsum)
                        pn = wk1.tile([128, H * M], BF16, tag="pn")
                        for h in range(H):
                            nc.gpsimd.tensor_scalar_mul(
                                pn[:, h * M:(h + 1) * M], es[:, h, :],
                                drec[:, h:h + 1])

                        for j2 in range(2):
                            ptr = ps1.tile([128, 128], BF16, tag="tr", bufs=2)
                            nc.tensor.transpose(
                                ptr, pn[:, j2 * 128:(j2 + 1) * 128], ident)
                            nc.scalar.copy(
                                pT_all[:, j2, c * 128:(c + 1) * 128], ptr)

                        # --- accumulate W = gated.T @ [x | 1] ---
                        st, sp = (c == 0), (c == NCH - 1)
                        nc.tensor.matmul(ps_w[:, 0:512], lhsT=gc,
                                         rhs=xrow[:, c4, 0:512],
                                         start=st, stop=sp)
                        nc.tensor.matmul(ps_w[:, 512:1024], lhsT=gc,
                                         rhs=xrow[:, c4, 512:1024],
                                         start=st, stop=sp)
                        nc.tensor.matmul(ps_w[:, 1024:1025], lhsT=gc,
                                         rhs=ones_sb, start=st, stop=sp)

                # --- slot gate ---
                ssum = singles.tile([64, 1], F32)
                nc.scalar.copy(ssum, ps_w[:, 1024:1025])
                sg = singles.tile([64, 1], F32)
                nc.gpsimd.tensor_scalar_min(sg, ssum, 1.0)
                mv_bf = singles.tile([64, D], BF16)
                nc.scalar.mul(mv_bf, ps_w[:, 0:D], sg)

            # ------- phase boundary: v = mv @ wv.T, VF = blockdiag(v) @ G -------
            wvT_sb = singles.tile([128, DC, D], BF16)
            nc.sync.dma_start(wvT_sb, wvT_r)
            gT_sb = singles.tile([128, DC, D], BF16)
            nc.sync.dma_start(gT_sb, gT_r)
            w1T_sb = singles.tile([128, DC, D], BF16)
            nc.sync.dma_start(w1T_sb, w1T_r)
            mvT_sb = singles.tile([128, DC, 64], BF16)
            vblk = singles.tile([128, 2, 4, 128], BF16)
            nc.vector.memset(vblk, 0.0)
            v_bf = singles.tile([64, D], BF16)
            with tc.tile_pool(name="psB", bufs=1, space="PSUM") as psB:
                for dc in range(DC):
                    ptr2 = psB.tile([128, 128], BF16, tag="tr2", bufs=2)
                    nc.tensor.transpose(
                        ptr2[:, 0:64],
                        mv_bf[:, dc * 128:(dc + 1) * 128],
                        ident[0:64, 0:64],
                    )
                    nc.vector.tensor_copy(mvT_sb[:, dc, :], ptr2[:, 0:64])
                pv = psB.tile([64, D], F32, tag="v")
                for g2 in range(2):
                    for dc in range(DC):
                        nc.tensor.matmul(
                            pv[:, g2 * 512:(g2 + 1) * 512],
                            lhsT=mvT_sb[:, dc, :],
                            rhs=wvT_sb[:, dc, g2 * 512:(g2 + 1) * 512],
                            start=(dc == 0), stop=(dc == DC - 1),
                        )
                nc.scalar.copy(v_bf, pv)
                # vT chunks placed into the zero-padded block-diag lhsT layout:
                # out-chunk j2 holds heads {2*j2, 2*j2+1}; contraction chunk i of
                # j2 is global d-chunk 4*j2+i (head 2*j2 + i//2) and lands in
                # column half i//2.
                for dc in range(DC):
                    ptr3 = psB.tile([128, 128], BF16, tag="tr2", bufs=2)
                    nc.tensor.transpose(
                        ptr3[:, 0:64],
                        v_bf[:, dc * 128:(dc + 1) * 128],
                        ident[0:64, 0:64],
                    )
                    j2, i = dc // 4, dc % 4
                    half = i // 2
                    nc.vector.tensor_copy(
                        vblk[:, j2, i, half * 64:(half + 1) * 64],
                        ptr3[:, 0:64])
                for j2 in range(2):
                    pvf = psB.tile([128, D], F32, tag="vf", bufs=1)
                    for g2 in range(2):
                        for i in range(4):
                            nc.tensor.matmul(
                                pvf[:, g2 * 512:(g2 + 1) * 512],
                                lhsT=vblk[:, j2, i, :],
                                rhs=gT_sb[:, 4 * j2 + i, g2 * 512:(g2 + 1) * 512],
                                start=(i == 0), stop=(i == 3),
                            )
                    nc.vector.tensor_copy(vf_sb[:, j2, :], pvf)

            # ---------------- phase 2: out = x @ W1 + P @ VF (+ combb) --------
            with (
                tc.tile_pool(name="ps2", bufs=1, space="PSUM") as ps2,
                tc.tile_pool(name="wk2", bufs=3) as wk2,
            ):
                for c in range(NCH):
                    osb = wk2.tile([128, D], F32, tag="osb")
                    for gh in range(2):
                        pf = ps2.tile([128, 512], F32, tag="f", bufs=4)
                        for dc in range(DC):
                            nc.tensor.matmul(
                                pf,
                                lhsT=xT_chunk(dc, c),
                                rhs=w1T_sb[:, dc, gh * 512:(gh + 1) * 512],
                                start=(dc == 0), stop=False,
                            )
                        for j2 in range(2):
                            nc.tensor.matmul(
                                pf,
                                lhsT=pT_all[:, j2, c * 128:(c + 1) * 128],
                                rhs=vf_sb[:, j2, gh * 512:(gh + 1) * 512],
                                start=False, stop=(j2 == 1),
                            )
                        dst = osb[:, gh * 512:(gh + 1) * 512]
                        if add_combb:
                            nc.vector.tensor_add(
                                dst, pf, combb_sb[:, gh * 512:(gh + 1) * 512])
                        elif gh == 0:
                            nc.scalar.copy(dst, pf)
                        else:
                            nc.vector.tensor_copy(dst, pf)
                    nc.sync.dma_start(
                        y_ap[c * 128:(c + 1) * 128, :], osb)

    nc.compile()
    return nc


def prep_inputs(inputs, S=4096):
    """Host-side fusion + per-core shard maps."""
    f64 = np.float64
    bf = ml_dtypes.bfloat16
    x = np.asarray(inputs["x"], np.float32)
    mk = np.asarray(inputs["memory_keys"], np.float32)
    wg_w = np.asarray(inputs["wg_w"], np.float32)
    wg_b = np.asarray(inputs["wg_b"], np.float32)
    ipw = np.asarray(inputs["in_proj_w"], np.float32)
    ipb = np.asarray(inputs["in_proj_b"], np.float32)
    out_w = np.asarray(inputs["out_w"], np.float32)
    out_b = np.asarray(inputs["out_b"], np.float32)
    comb_w = np.asarray(inputs["comb_w"], np.float32)
    comb_b = np.asarray(inputs["comb_b"], np.float32)

    wq, wk, wv = ipw[:D], ipw[D:2 * D], ipw[2 * D:]
    bq, bk, bv = ipb[:D], ipb[D:2 * D], ipb[2 * D:]

    k_full = mk.astype(f64) @ wk.astype(f64).T + bk.astype(f64)      # (M, D)
    kh = k_full.reshape(M, H, DH)
    wqh = wq.astype(f64).reshape(H, DH, D)
    scl = 1.0 / np.sqrt(DH)
    FK = (np.einsum("mhd,hde->hme", kh, wqh) * scl).reshape(H * M, D)
    sbias = (np.einsum("hd,mhd->hm", bq.astype(f64).reshape(H, DH), kh)
             * scl).reshape(H * M)
    BIG_W = np.concatenate([mk.astype(f64), wg_w.astype(f64), FK], axis=0)

    fused2 = comb_w[:, D:].astype(f64) @ out_w.astype(f64)           # (D, D)
    G = fused2.T                                                     # (D, D)
    combb = (comb_b.astype(f64) + comb_w[:, D:].astype(f64) @ out_b.astype(f64)
             + bv.astype(f64) @ G)

    DC = D // 128
    NT = S // 512

    def swz(w):
        # (D, E) -> (128, DC, E): row p holds [w[dc*128+p, :] for dc]
        e = w.shape[1]
        return np.ascontiguousarray(
            w.reshape(DC, 128, e).transpose(1, 0, 2)).astype(bf)

    shared = {
        "bigwT": swz(BIG_W.T),
        "w1T": swz(comb_w[:, :D].astype(f64).T),
        "wvT": swz(wv.astype(f64).T),
        "gmat": swz(G),
        "combb": combb.astype(np.float32),
        "wgbn": (-wg_b).astype(np.float32),
        "sbias": sbias.astype(np.float32),
    }
    add_sbias = bool(np.any(shared["sbias"] != 0))
    add_combb = bool(np.any(shared["combb"] != 0))

    in_maps = []
    for b in range(B):
        xb = x[b, :S]
        m = dict(shared)
        # (128, NT, 4, D): row-major x chunks, contiguous per partition
        m["x"] = np.ascontiguousarray(
            xb.reshape(NT, 4, 128, D).transpose(2, 0, 1, 3)).astype(bf)
        # (128, NT, DC, 512): x^T slices, contiguous per partition
        m["xT"] = np.ascontiguousarray(
            xb.T.reshape(DC, 128, NT, 512).transpose(1, 2, 0, 3)).astype(bf)
        in_maps.append(m)
    return in_maps, add_sbias, add_combb


def kernel(_trace=False, _S=4096, **inputs):
    in_maps, add_sbias, add_combb = prep_inputs(inputs, S=_S)
    nc = build_program(S=_S, add_sbias=add_sbias, add_combb=add_combb)
    kw = {}
    if _trace:
        kw = dict(trace=True, trace_cores=list(range(N_CORES)))
    res = run_bass_kernel_spmd(nc, in_maps, list(range(N_CORES)), **kw)
    y = np.stack([res.results[i]["y"] for i in range(N_CORES)], axis=0)
    y = y.astype(np.float32)
    if _trace:
        return y, res
    return y


# revision 18
# speedup vs baseline: 1.4962x; 1.4962x over previous
"""EpisodicMemory kernel for Trainium2, data-parallel over batch on 8 NeuronCores.

Per-core computation (one batch element b, S=4096, D=1024, M=64, H=4, DH=256):

Host-side algebraic fusion (exact linear algebra in fp64):
  k        = mk @ wk.T + bk                              (M, D)
  FUSED_K  = stack_h[(k_h @ wq_h) / sqrt(DH)]            (H*M, D)
  scores   = x @ FUSED_K.T + sbias        (replaces q-proj + qk matmul)
  BIG_W    = [mk | wg | FUSED_K]                         (M+1+H*M, D)
  fused2   = comb_w[:, D:] @ out_w                       (D, D);  G = fused2.T
  W1T      = comb_w[:, :D].T                             (D, D)
  combb    = comb_b + comb_w[:, D:] @ out_b + bv @ G     (D,)
  final    = x @ W1 + sum_h p_h @ VF_h + combb
  where VF_h = v_h @ G_h,  v = mv @ wv.T  (bv folded into combb since
  sum_m p_hm = 1), i.e. the old mem_out/out_w/comb2 GEMM chain collapses
  into one S x 256 x D matmul against a per-(head,slot) value table VF.

Device phases (per core):
  1. per s-chunk (128 rows): pbig = x_chunk @ BIG_W.T -> [sim | gate | scores]
     softmax(sim)*sigmoid(gate) -> gated -> accumulate W = gated.T @ [x | 1]
     softmax(scores) per head -> p -> PE-transpose -> pT_all (resident)
     No max-subtraction anywhere (|sim|,|gate| < ~5, |scores| < ~0.1 for this
     problem's scale-0.02 weights); sigmoid computed via the Exp table
     (sigmoid(g) = 1/(1+exp(-g))) so the ACT table is loaded exactly once.
  2. slot_gate = min(colsum, 1), mv = slot_gate * W, v = mv @ wv.T,
     VF = blockdiag(v_h) @ G  (256, D)
  3. per s-chunk: out = x_chunk @ W1 + P_chunk @ VF (+ combb) -> DMA out
All matmuls run in bf16 inputs with fp32 PSUM accumulation.
"""

import numpy as np
import ml_dtypes

import concourse.bass as bass
import concourse.mybir as mybir
import concourse.tile as tile
from concourse import bacc
from concourse.bass_utils import run_bass_kernel_spmd
from concourse.masks import make_identity

F32 = mybir.dt.float32
BF16 = mybir.dt.bfloat16
AX = mybir.AxisListType.X
AF = mybir.ActivationFunctionType
ALU = mybir.AluOpType

B, D, M, H = 8, 1024, 64, 4
DH = D // H
GW = M + 1 + H * M  # 321 columns of BIG_W output: [sim 0:64 | gate 64 | scores 65:321]
N_CORES = 8


def build_program(S=4096, add_sbias=False, add_combb=False):
    NCH = S // 128   # s-chunks
    NT = S // 512    # x row-tile groups (4 chunks each)
    DC = D // 128    # d-chunks

    nc = bacc.Bacc(None, target_bir_lowering=False, debug=False)

    # all big inputs are pre-swizzled on the host so every DMA line is
    # contiguous per partition (128 descriptors/transfer, not 1024)
    x_d = nc.dram_tensor("x", [128, NT, 4, D], BF16, kind="ExternalInput")
    xT_d = nc.dram_tensor("xT", [128, NT, DC, 512], BF16, kind="ExternalInput")
    bigwT_d = nc.dram_tensor("bigwT", [128, DC, GW], BF16, kind="ExternalInput")
    w1T_d = nc.dram_tensor("w1T", [128, DC, D], BF16, kind="ExternalInput")
    wvT_d = nc.dram_tensor("wvT", [128, DC, D], BF16, kind="ExternalInput")
    gT_d = nc.dram_tensor("gmat", [128, DC, D], BF16, kind="ExternalInput")
    combb_d = nc.dram_tensor("combb", [D], F32, kind="ExternalInput")
    wgbn_d = nc.dram_tensor("wgbn", [1], F32, kind="ExternalInput")
    sbias_d = nc.dram_tensor("sbias", [H * M], F32, kind="ExternalInput")
    y_d = nc.dram_tensor("y", [S, D], F32, kind="ExternalOutput")

    x_ap = x_d.ap()
    y_ap = y_d.ap()
    xT_r = xT_d.ap()
    bigwT_r = bigwT_d.ap()
    w1T_r = w1T_d.ap()
    wvT_r = wvT_d.ap()
    gT_r = gT_d.ap()

    def bcast(ap, n):
        return bass.AP(tensor=ap.tensor, offset=ap.offset, ap=[[0, n]] + list(ap.ap))

    with tile.TileContext(nc) as tc:
        with tc.tile_pool(name="singles", bufs=1) as singles:
            # resident inputs. bigwT first on the sync queue (first matmul needs
            # it), then xT in 512-column slices as separate tiles so early
            # chunks don't wait on the whole 8.4 MB transfer. wvT/gT/w1T are
            # deferred to just before their consumers (below) so they don't
            # delay the x stream on the sync queue.
            bigwT_sb = singles.tile([128, DC, GW], BF16)
            nc.sync.dma_start(bigwT_sb, bigwT_r)
            xT_t = []
            for t in range(NT):
                xts = singles.tile([128, DC, 512], BF16, tag=f"xts{t}")
                nc.sync.dma_start(xts, xT_r[:, t, :, :])
                xT_t.append(xts)

            def xT_chunk(dc, c):
                return xT_t[c // 4][:, dc, (c % 4) * 128:(c % 4 + 1) * 128]

            wgbn_sb = singles.tile([128, 1], F32)
            nc.gpsimd.dma_start(wgbn_sb, bcast(wgbn_d.ap(), 128))
            if add_sbias:
                sbias_sb = singles.tile([128, H * M], F32)
                nc.gpsimd.dma_start(sbias_sb, bcast(sbias_d.ap(), 128))
            if add_combb:
                combb_sb = singles.tile([128, D], F32)
                nc.gpsimd.dma_start(combb_sb, bcast(combb_d.ap(), 128))
            ident = singles.tile([128, 128], BF16)
            make_identity(nc, ident)
            ones_sb = singles.tile([128, 1], BF16)
            nc.vector.memset(ones_sb, 1.0)
            # residents produced by phase 1 / boundary
            pT_all = singles.tile([128, 2, S], BF16)
            vf_sb = singles.tile([128, 2, D], BF16)

            # ---------------- phase 1: write-attention ----------------
            with (
                tc.tile_pool(name="ps1", bufs=1, space="PSUM") as ps1,
                tc.tile_pool(name="xin", bufs=3) as xin,
                tc.tile_pool(name="wk1", bufs=3) as wk1,
            ):
                ps_w = ps1.tile([64, 1536], F32, tag="w")
                for t in range(NT):
                    xrow = xin.tile([128, 4, D], BF16, tag="xrow")
                    nc.gpsimd.dma_start(xrow, x_ap[:, t, :, :])
                    for c4 in range(4):
                        c = t * 4 + c4
                        pbig = ps1.tile([128, GW], F32, tag="big", bufs=3)
                        for dc in range(DC):
                            nc.tensor.matmul(
                                pbig,
                                lhsT=xT_chunk(dc, c),
                                rhs=bigwT_sb[:, dc, :],
                                start=(dc == 0), stop=(dc == DC - 1),
                            )
                        if add_sbias:
                            nc.vector.tensor_add(
                                pbig[:, M + 1:GW], pbig[:, M + 1:GW], sbias_sb
                            )

                        # --- write gate: softmax(sim) * sigmoid(gate) ---
                        # (no max subtraction; sigmoid via Exp table)
                        esum = wk1.tile([128, 1], F32, tag="esum")
                        e_sb = wk1.tile([128, M], F32, tag="esb")
                        nc.scalar.activation(e_sb, pbig[:, 0:M], AF.Exp,
                                             accum_out=esum)
                        tg = wk1.tile([128, 1], F32, tag="tg")
                        nc.scalar.activation(tg, pbig[:, M:M + 1], AF.Exp,
                                             scale=-1.0, bias=wgbn_sb)
                        u = wk1.tile([128, 1], F32, tag="u")
                        nc.vector.scalar_tensor_tensor(
                            u, tg, 1.0, esum, op0=ALU.add, op1=ALU.mult)
                        scale = wk1.tile([128, 1], F32, tag="scale")
                        nc.vector.reciprocal(scale, u)
                        gc = wk1.tile([128, M], BF16, tag="gc")
                        nc.vector.tensor_scalar_mul(gc, e_sb, scale)

                        # --- read attention probs, per head ---
                        es = wk1.tile([128, H, M], F32, tag="es")
                        nc.scalar.activation(es, pbig[:, M + 1:GW], AF.Exp)
                        dsum = wk1.tile([128, H], F32, tag="dsum")
                        nc.vector.reduce_sum(dsum, es, axis=AX)
                        drec = wk1.tile([128, H], F32, tag="drec")
                        nc.vector.reciprocal(drec, dsum)
                        pn = wk1.tile([128, H * M], BF16, tag="pn")
                        for h in range(H):
                            nc.vector.tensor_scalar_mul(
                                pn[:, h * M:(h + 1) * M], es[:, h, :],
                                drec[:, h:h + 1])

                        for j2 in range(2):
                            ptr = ps1.tile([128, 128], BF16, tag="tr", bufs=2)
                            nc.tensor.transpose(
                                ptr, pn[:, j2 * 128:(j2 + 1) * 128], ident)
                            nc.scalar.copy(
                                pT_all[:, j2, c * 128:(c + 1) * 128], ptr)

                        # --- accumulate W = gated.T @ [x | 1] ---
                        st, sp = (c == 0), (c == NCH - 1)
                        nc.tensor.matmul(ps_w[:, 0:512], lhsT=gc,
                                         rhs=xrow[:, c4, 0:512],
                                         start=st, stop=sp)
                        nc.tensor.matmul(ps_w[:, 512:1024], lhsT=gc,
                                         rhs=xrow[:, c4, 512:1024],
                                         start=st, stop=sp)
                        nc.tensor.matmul(ps_w[:, 1024:1025], lhsT=gc,
                                         rhs=ones_sb, start=st, stop=sp)

                # --- slot gate ---
                ssum = singles.tile([64, 1], F32)
                nc.scalar.copy(ssum, ps_w[:, 1024:1025])
                sg = singles.tile([64, 1], F32)
                nc.gpsimd.tensor_scalar_min(sg, ssum, 1.0)
                mv_bf = singles.tile([64, D], BF16)
                nc.scalar.mul(mv_bf, ps_w[:, 0:D], sg)

            # ------- phase boundary: v = mv @ wv.T, VF = blockdiag(v) @ G -------
            wvT_sb = singles.tile([128, DC, D], BF16)
            nc.sync.dma_start(wvT_sb, wvT_r)
            gT_sb = singles.tile([128, DC, D], BF16)
            nc.sync.dma_start(gT_sb, gT_r)
            w1T_sb = singles.tile([128, DC, D], BF16)
            nc.sync.dma_start(w1T_sb, w1T_r)
            mvT_sb = singles.tile([128, DC, 64], BF16)
            vblk = singles.tile([128, 2, 4, 128], BF16)
            nc.vector.memset(vblk, 0.0)
            v_bf = singles.tile([64, D], BF16)
            with tc.tile_pool(name="psB", bufs=1, space="PSUM") as psB:
                for dc in range(DC):
                    ptr2 = psB.tile([128, 128], BF16, tag="tr2", bufs=2)
                    nc.tensor.transpose(
                        ptr2[:, 0:64],
                        mv_bf[:, dc * 128:(dc + 1) * 128],
                        ident[0:64, 0:64],
                    )
                    nc.vector.tensor_copy(mvT_sb[:, dc, :], ptr2[:, 0:64])
                pv = psB.tile([64, D], F32, tag="v")
                for g2 in range(2):
                    for dc in range(DC):
                        nc.tensor.matmul(
                            pv[:, g2 * 512:(g2 + 1) * 512],
                            lhsT=mvT_sb[:, dc, :],
                            rhs=wvT_sb[:, dc, g2 * 512:(g2 + 1) * 512],
                            start=(dc == 0), stop=(dc == DC - 1),
                        )
                nc.scalar.copy(v_bf, pv)
                # vT chunks placed into the zero-padded block-diag lhsT layout:
                # out-chunk j2 holds heads {2*j2, 2*j2+1}; contraction chunk i of
                # j2 is global d-chunk 4*j2+i (head 2*j2 + i//2) and lands in
                # column half i//2.
                for dc in range(DC):
                    ptr3 = psB.tile([128, 128], BF16, tag="tr2", bufs=2)
                    nc.tensor.transpose(
                        ptr3[:, 0:64],
                        v_bf[:, dc * 128:(dc + 1) * 128],
                        ident[0:64, 0:64],
                    )
                    j2, i = dc // 4, dc % 4
                    half = i // 2
                    nc.vector.tensor_copy(
                        vblk[:, j2, i, half * 64:(half + 1) * 64],
                        ptr3[:, 0:64])
                for j2 in range(2):
                    pvf = psB.tile([128, D], F32, tag="vf", bufs=1)
                    for g2 in range(2):
                        for i in range(4):
                            nc.tensor.matmul(
                                pvf[:, g2 * 512:(g2 + 1) * 512],
                                lhsT=vblk[:, j2, i, :],
                                rhs=gT_sb[:, 4 * j2 + i, g2 * 512:(g2 + 1) * 512],
                                start=(i == 0), stop=(i == 3),
                            )
                    nc.vector.tensor_copy(vf_sb[:, j2, :], pvf)

            # ---------------- phase 2: out = x @ W1 + P @ VF (+ combb) --------
            with (
                tc.tile_pool(name="ps2", bufs=1, space="PSUM") as ps2,
                tc.tile_pool(name="wk2", bufs=3) as wk2,
            ):
                for c in range(NCH):
                    osb = wk2.tile([128, D], F32, tag="osb")
                    for gh in range(2):
                        pf = ps2.tile([128, 512], F32, tag="f", bufs=4)
                        for dc in range(DC):
                            nc.tensor.matmul(
                                pf,
                                lhsT=xT_chunk(dc, c),
                                rhs=w1T_sb[:, dc, gh * 512:(gh + 1) * 512],
                                start=(dc == 0), stop=False,
                            )
                        for j2 in range(2):
                            nc.tensor.matmul(
                                pf,
                                lhsT=pT_all[:, j2, c * 128:(c + 1) * 128],
                                rhs=vf_sb[:, j2, gh * 512:(gh + 1) * 512],
                                start=False, stop=(j2 == 1),
                            )
                        dst = osb[:, gh * 512:(gh + 1) * 512]
                        if add_combb:
                            nc.vector.tensor_add(
                                dst, pf, combb_sb[:, gh * 512:(gh + 1) * 512])
                        elif gh == 0:
                            nc.scalar.copy(dst, pf)
                        else:
                            nc.vector.tensor_copy(dst, pf)
                    nc.sync.dma_start(
                        y_ap[c * 128:(c + 1) * 128, :], osb)

    nc.compile()
    return nc


def prep_inputs(inputs, S=4096):
    """Host-side fusion + per-core shard maps."""
    f64 = np.float64
    bf = ml_dtypes.bfloat16
    x = np.asarray(inputs["x"], np.float32)
    mk = np.asarray(inputs["memory_keys"], np.float32)
    wg_w = np.asarray(inputs["wg_w"], np.float32)
    wg_b = np.asarray(inputs["wg_b"], np.float32)
    ipw = np.asarray(inputs["in_proj_w"], np.float32)
    ipb = np.asarray(inputs["in_proj_b"], np.float32)
    out_w = np.asarray(inputs["out_w"], np.float32)
    out_b = np.asarray(inputs["out_b"], np.float32)
    comb_w = np.asarray(inputs["comb_w"], np.float32)
    comb_b = np.asarray(inputs["comb_b"], np.float32)

    wq, wk, wv = ipw[:D], ipw[D:2 * D], ipw[2 * D:]
    bq, bk, bv = ipb[:D], ipb[D:2 * D], ipb[2 * D:]

    k_full = mk.astype(f64) @ wk.astype(f64).T + bk.astype(f64)      # (M, D)
    kh = k_full.reshape(M, H, DH)
    wqh = wq.astype(f64).reshape(H, DH, D)
    scl = 1.0 / np.sqrt(DH)
    FK = (np.einsum("mhd,hde->hme", kh, wqh) * scl).reshape(H * M, D)
    sbias = (np.einsum("hd,mhd->hm", bq.astype(f64).reshape(H, DH), kh)
             * scl).reshape(H * M)
    BIG_W = np.concatenate([mk.astype(f64), wg_w.astype(f64), FK], axis=0)

    fused2 = comb_w[:, D:].astype(f64) @ out_w.astype(f64)           # (D, D)
    G = fused2.T                                                     # (D, D)
    combb = (comb_b.astype(f64) + comb_w[:, D:].astype(f64) @ out_b.astype(f64)
             + bv.astype(f64) @ G)

    DC = D // 128
    NT = S // 512

    def swz(w):
        # (D, E) -> (128, DC, E): row p holds [w[dc*128+p, :] for dc]
        e = w.shape[1]
        return np.ascontiguousarray(
            w.reshape(DC, 128, e).transpose(1, 0, 2)).astype(bf)

    shared = {
        "bigwT": swz(BIG_W.T),
        "w1T": swz(comb_w[:, :D].astype(f64).T),
        "wvT": swz(wv.astype(f64).T),
        "gmat": swz(G),
        "combb": combb.astype(np.float32),
        "wgbn": (-wg_b).astype(np.float32),
        "sbias": sbias.astype(np.float32),
    }
    add_sbias = bool(np.any(shared["sbias"] != 0))
    add_combb = bool(np.any(shared["combb"] != 0))

    in_maps = []
    for b in range(B):
        xb = x[b, :S]
        m = dict(shared)
        # (128, NT, 4, D): row-major x chunks, contiguous per partition
        m["x"] = np.ascontiguousarray(
            xb.reshape(NT, 4, 128, D).transpose(2, 0, 1, 3)).astype(bf)
        # (128, NT, DC, 512): x^T slices, contiguous per partition
        m["xT"] = np.ascontiguousarray(
            xb.T.reshape(DC, 128, NT, 512).transpose(1, 2, 0, 3)).astype(bf)
        in_maps.append(m)
    return in_maps, add_sbias, add_combb


def kernel(_trace=False, _S=4096, **inputs):
    in_maps, add_sbias, add_combb = prep_inputs(inputs, S=_S)
    nc = build_program(S=_S, add_sbias=add_sbias, add_combb=add_combb)
    kw = {}
    if _trace:
        kw = dict(trace=True, trace_cores=list(range(N_CORES)))
    res = run_bass_kernel_spmd(nc, in_maps, list(range(N_CORES)), **kw)
    y = np.stack([res.results[i]["y"] for i in range(N_CORES)], axis=0)
    y = y.astype(np.float32)
    if _trace:
        return y, res
    return y
